# revision 3
# baseline (speedup 1.0000x reference)
"""AttentionWithMemory on 8 Trainium2 NeuronCores (Bass/Tile).

Sharding (classic distributed KNN, per the hint):
  - query rows (B*S = 4096) split 8 ways: core c owns rows [512c, 512(c+1)),
    all inside batch b = c // 4;
  - memory bank rows (M = 16384) split 8 ways: core c holds the pre-normalized
    transposed shard mk_nT[:, 2048c : 2048(c+1)] and computes local top-8 for
    ALL 4096 queries (canonical row order = owner-major); candidates, packed
    as (truncated-value | bit-reversed-global-index) in fp32 bits, are
    exchanged with one AllToAll so each core receives the 8 shard-local
    candidate lists for exactly its own rows, merged with a single max8;
  - memory values are replicated in HBM for the indirect row gather.

Joint softmax over [S self | K memory] columns is decomposed:
  ctx[h,q] = (sum_k exp(s_hqk) v_k + sum_j exp(m_qj) mv_j)
             / (Zself[h,q] + Zmem[q])
Self part runs transposed (scoresT -> expT -> ctxT accumulation) with a ones
column appended per head in v so row 64 of the ctxT accumulator is Zself.
The head-independent memory numerator is built q-major from the gather and
PE-transposed to e-major; assembly and the output projection stay in [e, q]
orientation, producing the [512, 1024] row shard directly.
"""

import contextlib

import numpy as np

import concourse.bass as bass
import concourse.mybir as mybir
import concourse.tile as tile
from concourse.bass import IndirectOffsetOnAxis
from concourse.masks import make_identity

F32 = mybir.dt.float32
F16 = mybir.dt.float16
BF16 = mybir.dt.bfloat16
U32 = mybir.dt.uint32
U16 = mybir.dt.uint16
AF = mybir.ActivationFunctionType
OP = mybir.AluOpType

N_CORES = 8
B, S, E, M = 2, 2048, 1024, 16384
H = 16
HD = E // H               # 64
QR = (B * S) // N_CORES   # 512 own query rows per core
MS = M // N_CORES         # 2048 memory rows per core
K = 8
P = 128
EPS = 1e-12

N_ET = E // P             # 8 e-tiles
N_QT = QR // P            # 4 own query tiles
N_SQT = (B * S) // P      # 32 sims query tiles
N_KT = S // P             # 16 key tiles per batch
SCALE = 1.0 / float(np.sqrt(np.float32(HD)))

# candidate encoding: raw sims (mk_n rows unit-norm, x rows ~N(0,1)) are
# ~N(0,1); raw*ENC_A + ENC_B lands in (2, 4) = one fp32 octave, so the
# 14-bit-truncated value ORed with the bit-reversed global index compares
# monotonically (value first, lowest index wins ties, matching jax top_k).
ENC_A = 0.12
ENC_B = 3.0
IDX_MASK = 0x3FFF
VAL_MASK = 0xFFFFC000


def build_nc():
    nc = bass.Bass()
    io = {}
    dp = nc.declare_dram_parameter
    io["xT"] = dp("xT", [E, B * S], F16, isOutput=False)
    io["xTb"] = dp("xTb", [E, S], F16, isOutput=False)
    io["mknT"] = dp("mknT", [E, MS], F16, isOutput=False)
    io["mv"] = dp("mv", [M, E], F16, isOutput=False)
    io["wqT"] = dp("wqT", [E, E], F16, isOutput=False)
    io["wkT"] = dp("wkT", [E, E], F16, isOutput=False)
    io["wvT"] = dp("wvT", [E, E], F16, isOutput=False)
    io["woT"] = dp("woT", [E, E], F16, isOutput=False)
    io["bqt"] = dp("bqt", [P, N_ET], F32, isOutput=False)
    io["bkt"] = dp("bkt", [P, N_ET], F32, isOutput=False)
    io["bvr"] = dp("bvr", [1, E], F16, isOutput=False)
    io["bor"] = dp("bor", [1, E], F16, isOutput=False)
    io["expaff"] = dp("expaff", [QR, 2], F32, isOutput=False)
    io["uconst"] = dp("uconst", [P, 1], U32, isOutput=False)
    io["out"] = dp("out", [QR, E], F32, isOutput=True)

    with tile.TileContext(nc) as tc:
        _body(nc, tc, io)
    return nc


def _body(nc, tc, io):
    ctx = contextlib.ExitStack()
    with ctx:
        pool_in = ctx.enter_context(tc.tile_pool(name="inputs", bufs=1))
        pool_str = ctx.enter_context(tc.tile_pool(name="stream", bufs=16))
        pool_r = ctx.enter_context(tc.tile_pool(name="retr", bufs=1))
        pool_rw = ctx.enter_context(tc.tile_pool(name="retr_w", bufs=2))
        pool_a = ctx.enter_context(tc.tile_pool(name="attn", bufs=1))
        pool_aw = ctx.enter_context(tc.tile_pool(name="attn_w", bufs=3))
        pool_w5 = ctx.enter_context(tc.tile_pool(name="wblk512", bufs=9))
        pool_kt = ctx.enter_context(tc.tile_pool(name="ktpool", bufs=2))
        pool_asm = ctx.enter_context(tc.tile_pool(name="asm", bufs=2))
        psum_mm = ctx.enter_context(
            tc.tile_pool(name="psum_mm", bufs=4, space="PSUM"))
        psum_ctx = ctx.enter_context(
            tc.tile_pool(name="psum_ctx", bufs=2, space="PSUM"))
        psum_tr = ctx.enter_context(
            tc.tile_pool(name="psum_tr", bufs=1, space="PSUM"))
        dram = ctx.enter_context(tc.tile_pool(name="dram", bufs=1, space="DRAM"))

        # ---------------- resident inputs ----------------
        mkn_sb = []
        for et in range(N_ET):
            mt = pool_in.tile([P, MS], F16, tag=f"mknT{et}")
            nc.sync.dma_start(mt[:], io["mknT"][et * P:(et + 1) * P, :])
            mkn_sb.append(mt)
        xTb_sb = []
        for et in range(N_ET):
            xt = pool_in.tile([P, S], F16, tag=f"xTb{et}")
            nc.sync.dma_start(xt[:], io["xTb"][et * P:(et + 1) * P, :])
            xTb_sb.append(xt)
        bq_sb = pool_in.tile([P, N_ET], F32, tag="bq")
        nc.sync.dma_start(bq_sb[:], io["bqt"][:])
        bk_sb = pool_in.tile([P, N_ET], F32, tag="bk")
        nc.sync.dma_start(bk_sb[:], io["bkt"][:])
        bv_sb = pool_in.tile([1, E], F16, tag="bv")
        nc.sync.dma_start(bv_sb[:], io["bvr"][:])
        bo_sb = pool_in.tile([1, E], F16, tag="bo")
        nc.sync.dma_start(bo_sb[:], io["bor"][:])
        uc_sb = pool_in.tile([P, 1], U32, tag="uconst")
        nc.sync.dma_start(uc_sb[:], io["uconst"][:])
        aff_sb = []
        for qt in range(N_QT):
            af = pool_in.tile([P, 2], F32, tag=f"aff{qt}")
            nc.sync.dma_start(af[:], io["expaff"][qt * P:(qt + 1) * P, :])
            aff_sb.append(af)

        ident_b = pool_in.tile([P, P], F16, tag="identb")
        make_identity(nc, ident_b[:])
        ident_f = pool_in.tile([P, P], F32, tag="identf")
        make_identity(nc, ident_f[:])
        ones_b = pool_in.tile([1, E], F16, tag="onesb")
        nc.vector.memset(ones_b[:], 1.0)
        ones_f = pool_in.tile([1, P], F32, tag="onesf")
        nc.vector.memset(ones_f[:], 1.0)

        # ---------------- PHASE R: distributed KNN ----------------
        cand_local = dram.tile([B * S, K], U32)
        for qt in range(N_SQT):
            sims_sb = pool_rw.tile([P, MS], F16, tag="sims")
            lhs = []
            for et in range(N_ET):
                lt = pool_str.tile([P, P], F16, tag="simlhs")
                nc.sync.dma_start(
                    lt[:], io["xT"][et * P:(et + 1) * P, qt * P:(qt + 1) * P])
                lhs.append(lt)
            for mc in range(MS // 512):
                ps = psum_mm.tile([P, 512], F32, tag="mm")
                for et in range(N_ET):
                    nc.tensor.matmul(
                        ps[:], lhs[et][:],
                        mkn_sb[et][:, mc * 512:(mc + 1) * 512],
                        start=(et == 0), stop=(et == N_ET - 1))
                nc.scalar.activation(
                    sims_sb[:, mc * 512:(mc + 1) * 512], ps[:], AF.Copy)
            v8 = pool_rw.tile([P, K], F16, tag="v8")
            nc.vector.max(out=v8[:], in_=sims_sb[:])
            li = pool_rw.tile([P, K], U32, tag="li")
            nc.vector.max_index(out=li[:], in_max=v8[:], in_values=sims_sb[:])
            # global idx: local idx (<2048) OR core-offset (multiple of 2048)
            # equals addition; then bit-reverse within 14 bits via XOR.
            gi = pool_rw.tile([P, K], U32, tag="gi")
            nc.vector.tensor_scalar(gi[:], li[:], uc_sb[:, 0:1], None,
                                    op0=OP.bitwise_or)
            nc.vector.tensor_scalar(gi[:], gi[:], IDX_MASK, None,
                                    op0=OP.bitwise_xor)
            eraw = pool_rw.tile([P, K], F16, tag="eraw")
            nc.scalar.activation(eraw[:], v8[:], AF.Exp)
            e32 = pool_rw.tile([P, K], U32, tag="e32")
            nc.vector.tensor_copy(e32[:], eraw[:].bitcast(U16))
            nc.vector.tensor_scalar(e32[:], e32[:], 14, None,
                                    op0=OP.logical_shift_left)
            enc = pool_rw.tile([P, K], U32, tag="enc")
            nc.vector.tensor_tensor(enc[:], e32[:], gi[:], op=OP.bitwise_or)
            nc.sync.dma_start(cand_local[qt * P:(qt + 1) * P, :], enc[:])

        cand_mine = dram.tile([N_CORES, QR, K], U32)
        nc.gpsimd.collective_compute(
            "AllToAll", OP.bypass,
            replica_groups=[list(range(N_CORES))],
            ins=[cand_local.opt()],
            outs=[cand_mine.opt()],
        )

        nmem_sb = []
        zmem_sb = []
        for qt in range(N_QT):
            merge_in = pool_rw.tile([P, N_CORES * K], U32, tag="mrg")
            src = cand_mine[:, qt * P:(qt + 1) * P, :].rearrange(
                "c p k -> p c k")
            nc.sync.dma_start(
                merge_in[:].rearrange("p (c k) -> p c k", k=K), src)
            top8 = pool_rw.tile([P, K], F32, tag="top8")
            nc.vector.max(out=top8[:], in_=merge_in[:].bitcast(F32))
            gidx = pool_r.tile([P, K], U32, tag=f"gidx{qt}")
            nc.vector.tensor_scalar(gidx[:], top8[:].bitcast(U32), IDX_MASK,
                                    IDX_MASK, op0=OP.bitwise_and,
                                    op1=OP.bitwise_xor)
            vsh = pool_rw.tile([P, K], U32, tag="vsh")
            nc.vector.tensor_scalar(vsh[:], top8[:].bitcast(U32), 14, None,
                                    op0=OP.logical_shift_right)
            v16 = pool_rw.tile([P, K], U16, tag="v16")
            nc.vector.tensor_copy(v16[:], vsh[:])
            lnv = pool_rw.tile([P, K], F32, tag="lnv")
            nc.scalar.activation(lnv[:], v16[:].bitcast(F16), AF.Ln)
            w8 = pool_r.tile([P, K], F32, tag=f"w8{qt}")
            zm = pool_r.tile([P, 1], F32, tag=f"zm{qt}")
            nc.scalar.activation(
                w8[:], lnv[:], AF.Exp,
                scale=aff_sb[qt][:, 0:1], accum_out=zm[:])
            zmem_sb.append(zm)
            acc = pool_r.tile([P, E], F16, tag=f"nmem{qt}")
            for j in range(K):
                g = pool_rw.tile([P, E], F16, tag="gath")
                nc.gpsimd.indirect_dma_start(
                    out=g[:], out_offset=None, in_=io["mv"][:],
                    in_offset=IndirectOffsetOnAxis(
                        ap=gidx[:, j:j + 1], axis=0))
                if j == 0:
                    nc.vector.tensor_scalar_mul(acc[:], g[:], w8[:, 0:1])
                else:
                    tmp = pool_rw.tile([P, E], F16, tag="gtmp")
                    nc.vector.tensor_scalar_mul(tmp[:], g[:], w8[:, j:j + 1])
                    nc.vector.tensor_add(acc[:], acc[:], tmp[:])
            nmem_sb.append(acc)

        nmemT_sb = []
        for _et in range(N_ET):
            nmt = pool_r.tile([P, QR], F16, tag=f"nmemT{_et}", name=f"nmemT{_et}")
            nmemT_sb.append(nmt)
        zmemT_sb = pool_r.tile([1, QR], F32, tag="zmemT")
        for qt in range(N_QT):
            for et in range(N_ET):
                pst = psum_tr.tile([P, P], F16, tag="tr")
                nc.tensor.transpose(
                    out=pst[:], in_=nmem_sb[qt][:, et * P:(et + 1) * P],
                    identity=ident_b[:])
                nc.scalar.activation(
                    nmemT_sb[et][:, qt * P:(qt + 1) * P], pst[:], AF.Copy)
            zblk = pool_rw.tile([P, P], F32, tag="zblk")
            nc.vector.memset(zblk[:], 0.0)
            nc.vector.tensor_copy(zblk[:, 0:1], zmem_sb[qt][:])
            pst = psum_tr.tile([P, P], F32, tag="tr")
            nc.tensor.transpose(out=pst[:], in_=zblk[:], identity=ident_f[:])
            nc.vector.tensor_copy(
                zmemT_sb[:, qt * P:(qt + 1) * P], pst[0:1, :])

        # ---------------- PHASE A: attention ----------------
        # own query rows sit at xTb columns [0, QR) (host rotates the batch;
        # key order is irrelevant since attention sums over keys).

        # qT [E, 512]
        qT_sb = []
        for ot in range(N_ET):
            ps = psum_mm.tile([P, 512], F32, tag="mm")
            for et in range(N_ET):
                wt = pool_str.tile([P, P], F16, tag="wqblk")
                nc.sync.dma_start(
                    wt[:], io["wqT"][et * P:(et + 1) * P, ot * P:(ot + 1) * P])
                nc.tensor.matmul(ps[:], wt[:], xTb_sb[et][:, 0:QR],
                                 start=(et == 0), stop=(et == N_ET - 1))
            qt_t = pool_a.tile([P, QR], F16, tag=f"qT{ot}")
            nc.scalar.activation(qt_t[:], ps[:], AF.Identity,
                                 bias=bq_sb[:, ot:ot + 1])
            qT_sb.append(qt_t)

        # v_aug [keys, 16*(64+1)] per key tile; wv streamed per half
        v_sb = []
        for rt in range(N_KT):
            vt = pool_a.tile([P, H * (HD + 1)], F16, tag=f"vaug{rt}")
            v_sb.append(vt)
        for oc in range(2):
            wvc = []
            for et in range(N_ET):
                wt = pool_w5.tile([P, 512], F16, tag="wblk512")
                nc.sync.dma_start(
                    wt[:],
                    io["wvT"][et * P:(et + 1) * P, oc * 512:(oc + 1) * 512])
                wvc.append(wt)
            for rt in range(N_KT):
                ps = psum_mm.tile([P, 512], F32, tag="mm")
                for et in range(N_ET):
                    nc.tensor.matmul(
                        ps[:], xTb_sb[et][:, rt * P:(rt + 1) * P], wvc[et][:],
                        start=(et == 0), stop=False)
                nc.tensor.matmul(ps[:], ones_b[:, 0:P],
                                 bv_sb[:, oc * 512:(oc + 1) * 512],
                                 start=False, stop=True)
                dst = v_sb[rt][:].rearrange("p (h c) -> p h c", c=HD + 1)
                nc.scalar.activation(
                    dst[:, oc * 8:(oc + 1) * 8, 0:HD],
                    ps[:].rearrange("p (h c) -> p h c", c=HD),
                    AF.Copy)
        for rt in range(N_KT):
            spots = v_sb[rt][:].rearrange("p (h c) -> p h c", c=HD + 1)
            nc.vector.memset(spots[:, :, HD:HD + 1], 1.0)

        # per head pair: kT, then per head scoresT/expT/ctxT + assembly
        ctxT_sb = []
        for _et in range(N_ET):
            ct = pool_a.tile([P, QR], F16, tag=f"ctxT{_et}", name=f"ctxT{_et}")
            ctxT_sb.append(ct)
        for ot in range(N_ET):
            kt_t = pool_kt.tile([P, S], F16, tag="kT")
            wkc = []
            for et in range(N_ET):
                wt = pool_str.tile([P, P], F16, tag="wkblk")
                nc.sync.dma_start(
                    wt[:],
                    io["wkT"][et * P:(et + 1) * P, ot * P:(ot + 1) * P])
                wkc.append(wt)
            for kc in range(4):
                ps = psum_mm.tile([P, 512], F32, tag="mm")
                for et in range(N_ET):
                    nc.tensor.matmul(
                        ps[:], wkc[et][:],
                        xTb_sb[et][:, kc * 512:(kc + 1) * 512],
                        start=(et == 0), stop=(et == N_ET - 1))
                nc.scalar.activation(kt_t[:, kc * 512:(kc + 1) * 512], ps[:],
                                     AF.Identity, bias=bk_sb[:, ot:ot + 1])
            for hh in range(2):
                h = 2 * ot + hh
                hof = hh * HD
                cps = psum_ctx.tile([HD + 1, 512], F32, tag="ctx")
                for kt in range(N_KT):
                    sps = psum_mm.tile([P, 512], F32, tag="mm")
                    nc.tensor.matmul(
                        sps[:], kt_t[hof:hof + HD, kt * P:(kt + 1) * P],
                        qT_sb[ot][hof:hof + HD, :],
                        start=True, stop=True)
                    ex = pool_aw.tile([P, 512], F16, tag="expT")
                    nc.scalar.activation(ex[:], sps[:], AF.Exp, scale=SCALE)
                    nc.tensor.matmul(
                        cps[:], v_sb[kt][:, h * (HD + 1):(h + 1) * (HD + 1)],
                        ex[:], start=(kt == 0), stop=(kt == N_KT - 1))
                # assembly for head h (cross-base copies are legal; the
                # tensor_tensor ops all run base-aligned at rof)
                et2, rof = divmod(h * HD, P)
                zden = pool_asm.tile([1, QR], F32, tag="zden")
                nc.vector.tensor_copy(zden[:], cps[HD:HD + 1, :])
                nc.vector.tensor_add(zden[:], zden[:], zmemT_sb[:])
                rec = pool_asm.tile([1, QR], F32, tag="rec")
                nc.vector.reciprocal(rec[:], zden[:])
                rb_ps = psum_tr.tile([HD, 512], F32, tag="rb")
                nc.tensor.matmul(rb_ps[:], ones_f[:, 0:HD], rec[:],
                                 start=True, stop=True)
                rb = pool_asm.tile([P, QR], F16, tag="rbsb")
                nc.scalar.activation(rb[rof:rof + HD, :], rb_ps[:], AF.Copy)
                rows = ctxT_sb[et2][rof:rof + HD, :]
                nc.scalar.activation(rows, cps[0:HD, :], AF.Copy)
                nc.vector.tensor_add(rows, rows,
                                     nmemT_sb[et2][rof:rof + HD, :])
                nc.vector.tensor_mul(rows, rows, rb[rof:rof + HD, :])

        # out projection: out[q, o] = ctxT.T @ woT + bo (wo streamed)
        for oc in range(2):
            woc = []
            for et in range(N_ET):
                wt = pool_w5.tile([P, 512], F16, tag="wblk512")
                nc.sync.dma_start(
                    wt[:],
                    io["woT"][et * P:(et + 1) * P, oc * 512:(oc + 1) * 512])
                woc.append(wt)
            for qt in range(N_QT):
                ps = psum_mm.tile([P, 512], F32, tag="mm")
                for et in range(N_ET):
                    nc.tensor.matmul(
                        ps[:], ctxT_sb[et][:, qt * P:(qt + 1) * P], woc[et][:],
                        start=(et == 0), stop=False)
                nc.tensor.matmul(ps[:], ones_b[:, 0:P],
                                 bo_sb[:, oc * 512:(oc + 1) * 512],
                                 start=False, stop=True)
                osb = pool_asm.tile([P, 512], F32, tag="osb")
                nc.scalar.activation(osb[:], ps[:], AF.Copy)
                nc.sync.dma_start(
                    io["out"][qt * P:(qt + 1) * P, oc * 512:(oc + 1) * 512],
                    osb[:])


def host_prep(inputs):
    """Produce the 8 per-core in_maps from the full numpy inputs."""
    hs = np.asarray(inputs["hidden_states"], dtype=np.float32)
    mk = np.asarray(inputs["memory_keys"], dtype=np.float32)
    mvf = np.asarray(inputs["memory_values"], dtype=np.float32)
    x = np.ascontiguousarray(hs.reshape(B * S, E))

    xT = np.ascontiguousarray(x.T).astype(np.float16)
    rn = np.maximum(np.linalg.norm(x, axis=1), EPS)
    mk_n = mk / np.maximum(np.linalg.norm(mk, axis=1, keepdims=True), EPS)
    mknT_full = np.ascontiguousarray(mk_n.T).astype(np.float16)
    mv_b = mvf.astype(np.float16)

    def wT(name):
        return np.ascontiguousarray(
            np.asarray(inputs[name], np.float32).T).astype(np.float16)

    wq, wk, wv, wo = wT("Wq"), wT("Wk"), wT("Wv"), wT("Wo")
    bq = np.asarray(inputs["bq"], np.float32)
    bk = np.asarray(inputs["bk"], np.float32)
    bqt = np.ascontiguousarray(bq.reshape(N_ET, P).T)
    bkt = np.ascontiguousarray(bk.reshape(N_ET, P).T)
    bvr = np.asarray(inputs["bv"], np.float32).reshape(1, E).astype(np.float16)
    bor = np.asarray(inputs["bo"], np.float32).reshape(1, E).astype(np.float16)

    in_maps = []
    for c in range(N_CORES):
        b = c // (N_CORES // B)
        rows = slice(c * QR, (c + 1) * QR)
        rinv = (1.0 / rn[rows]).astype(np.float32)
        aff = np.stack([rinv, np.zeros_like(rinv)], axis=1).astype(np.float32)
        w = (c % (N_CORES // B)) * QR
        xb = xT[:, b * S:(b + 1) * S]
        xb_rot = np.concatenate([xb[:, w:], xb[:, :w]], axis=1)
        in_maps.append({
            "xT": xT,
            "xTb": np.ascontiguousarray(xb_rot),
            "mknT": np.ascontiguousarray(mknT_full[:, c * MS:(c + 1) * MS]),
            "mv": mv_b,
            "wqT": wq, "wkT": wk, "wvT": wv, "woT": wo,
            "bqt": bqt, "bkt": bkt, "bvr": bvr, "bor": bor,
            "expaff": np.ascontiguousarray(aff),
            "uconst": np.full((P, 1), c * MS, np.uint32),
        })
    return in_maps


def assemble(results):
    shards = [np.asarray(results[c]["out"]) for c in range(N_CORES)]
    return np.concatenate(shards, axis=0).reshape(B, S, E).astype(np.float32)


# ---------------------------------------------------------------------------
# Workaround: walrus in this environment encodes at most ONE sync-wait per
# instruction.  (a) the TileContext exit drain waits on the whole global
# clock -> split across several SP drains; (b) any instruction given >1
# waits by the Tile scheduler gets the excess hoisted onto InstNoOp
# carriers inserted just before it on the same engine.
# ---------------------------------------------------------------------------
_MAX_W = 1
_tilefix_applied = False


def _split_sync_waits(nc, max_w: int = _MAX_W):
    n_split = 0
    for fn in nc.m.functions:
        for bb in fn.blocks:
            new_insts = []
            for inst in bb.instructions:
                si = inst.sync_info
                if si and si.on_wait and len(si.on_wait) > max_w:
                    waits = list(si.on_wait)
                    excess, keep = waits[:-max_w], waits[-max_w:]
                    for i in range(0, len(excess), max_w):
                        nop = mybir.InstNoOp(
                            name=f"{inst.name}_sw{i}",
                            engine=inst.engine,
                            sync_info=mybir.SyncInfo(
                                on_wait=excess[i : i + max_w], on_update=[]
                            ),
                        )
                        new_insts.append(nop)
                        n_split += 1
                    si.on_wait = keep
                new_insts.append(inst)
            bb.instructions[:] = new_insts
    return n_split


def _apply_tilefix():
    global _tilefix_applied
    if _tilefix_applied:
        return
    _tilefix_applied = True
    from concourse.vector_clock import ScopedClock

    def patched_dab(self, tick_clock, wait_clock):
        nc = self.nc
        drain_inst = nc.sync.drain()
        wait_clock.add_sem_waits(
            drain_inst.ins, ScopedClock({None: tick_clock.global_clock})
        )
        si = drain_inst.ins.sync_info
        waits = list(si.on_wait) if si and si.on_wait else []
        if len(waits) > _MAX_W:
            si.on_wait = waits[:_MAX_W]
            rest = waits[_MAX_W:]
            while rest:
                extra = nc.sync.drain()
                extra.ins.sync_info = mybir.SyncInfo(
                    on_wait=rest[:_MAX_W], on_update=[]
                )
                rest = rest[_MAX_W:]
        nc.all_engine_barrier()
        popped = nc._tile_sem_poison_stack.pop()
        assert popped is self._sem_poison
        nc.clear_and_free_semaphores(list(self.sems.allocated().values()))
        nc.all_engine_barrier()

    tile.TileContext._drain_and_barrier = patched_dab


# ---------------------------------------------------------------------------
# Runner: build once per process, execute on the 8 NeuronCores via the SPMD
# bass path, reassemble the full output.
# ---------------------------------------------------------------------------
_CACHE = {}


def _get_nc():
    if "nc" not in _CACHE:
        _apply_tilefix()
        nc = build_nc()
        _split_sync_waits(nc)
        _CACHE["nc"] = nc
    return _CACHE["nc"]


def kernel(**inputs):
    from concourse.bass_utils import run_bass_kernel_spmd

    nc = _get_nc()
    in_maps = host_prep(inputs)
    res = run_bass_kernel_spmd(nc, in_maps, list(range(N_CORES)))
    return assemble(res.results)


def timed_device_exec(inputs, reps: int = 6):
    """Time repeated executions of the compiled SPMD program with inputs
    resident on the 8 devices (min over reps, ns)."""
    import time

    import jax
    from concourse import bass2jax

    nc = _get_nc()
    in_maps = host_prep(inputs)
    key = "timed_fn"
    if key not in _CACHE:
        bass2jax.install_neuronx_cc_hook()
        partition_name = (
            nc.partition_id_tensor.name if nc.partition_id_tensor else None
        )
        in_names, out_names, out_avals, zero_outs = [], [], [], []
        for alloc in nc.m.functions[0].allocations:
            if not isinstance(alloc, mybir.MemoryLocationSet):
                continue
            name = alloc.memorylocations[0].name
            if alloc.kind == "ExternalInput":
                if name != partition_name:
                    in_names.append(name)
            elif alloc.kind == "ExternalOutput":
                shape = tuple(alloc.tensor_shape)
                dtype = mybir.dt.np(alloc.dtype)
                out_names.append(name)
                out_avals.append(jax.core.ShapedArray(shape, dtype))
                zero_outs.append(np.zeros(shape, dtype))
        n_params = len(in_names)
        all_names = in_names + out_names
        if partition_name is not None:
            all_names.append(partition_name)

        def _fbody(*args):
            operands = list(args)
            if partition_name is not None:
                operands.append(bass2jax.partition_id_tensor())
            outs = bass2jax._bass_exec_p.bind(
                *operands,
                out_avals=tuple(out_avals),
                in_names=tuple(all_names),
                out_names=tuple(out_names),
                lowering_input_output_aliases=(),
                sim_require_finite=True,
                sim_require_nnan=True,
                nc=nc,
            )
            return tuple(outs)

        devices = jax.devices()[:N_CORES]
        mesh = bass2jax.Mesh(np.asarray(devices), ("core",))
        pspec = bass2jax.PartitionSpec("core")
        n_all = n_params + len(out_names)
        jfn = jax.jit(
            bass2jax.shard_map(
                _fbody, mesh=mesh, in_specs=(pspec,) * n_all,
                out_specs=(pspec,) * len(out_names), check_rep=False,
            )
        )
        sharding = jax.sharding.NamedSharding(mesh, pspec)
        _CACHE[key] = (jfn, in_names, zero_outs, sharding)

    jfn, in_names, zero_outs, sharding = _CACHE[key]
    dev_args = [
        jax.device_put(
            np.concatenate([np.asarray(m[n]) for m in in_maps], axis=0),
            sharding,
        )
        for n in in_names
    ]
    dev_args += [
        jax.device_put(np.concatenate([z] * N_CORES, axis=0), sharding)
        for z in zero_outs
    ]
    outs = jfn(*dev_args)  # compile + warm
    jax.block_until_ready(outs)
    best = None
    for _ in range(reps):
        t0 = time.perf_counter()
        outs = jfn(*dev_args)
        jax.block_until_ready(outs)
        dt = time.perf_counter() - t0
        best = dt if best is None or dt < best else best
    return int(best * 1e9)
